# revision 13
# baseline (speedup 1.0000x reference)
"""Trainium2 Bass kernel for nn_DRM_Matching (topk_masking).

Reference semantics (per batch b, history h):
    scores[l] = <sel[b,h,l]/||sel[b,h,l]||, usr[b]/||usr[b]||>   (l = 0..127)
    vals, ids = top_k(scores, 16)            # descending
    out_w[b,h,j] = news[b,h,ids[j]] * (vals[j] if vals[j] >= thr else 0)
    out_idx[b,h,j] = ids[j]

Sharding: pure data-parallel over batch, 4 batches per core on 8 cores.

Per-core pipeline:
  - stream sel tiles [128L x 256D] from HBM (1.3 MB chunks)
  - DVE: dot(sel, usr_n) per row via scalar_tensor_tensor accum
  - ACT: sum(sel^2) per row via activation(Square, accum_out)
  - per batch: scores = dot / max(sqrt(ss), eps); PE-transpose to [50H, 128L];
    top-16 via DVE max8/max_index/match_replace (two rounds)
  - weights/global gather indices bounce through DRAM scratch to re-layout
    from [50,16] to [128,1] per-partition columns
  - gpsimd indirect DMA gathers only the 16/128 needed news rows; DVE applies
    weights; DMA out
"""

import numpy as np

import concourse.bacc as bacc
import concourse.bass as bass
import concourse.mybir as mybir
import concourse.tile as tile
from concourse.bass_utils import run_bass_kernel_spmd
from concourse.masks import make_identity

# Problem geometry (hardcoded per spec)
B, H, L, D = 32, 50, 128, 256
K = 16
THR = 0.1
EPS = 1e-12
NCORES = 8
BL = B // NCORES          # 4 local batches per core
NT = BL * H               # 200 (b,h) rows per core
NG = NT * K               # 3200 gathered rows per core
CH = 10                   # histories per sel DMA chunk (1.31 MB per DMA)
NEG = -1.0e30             # replacement value for found maxima

f32 = mybir.dt.float32
i32 = mybir.dt.int32
u32 = mybir.dt.uint32
AF = mybir.ActivationFunctionType
OP = mybir.AluOpType


def _emit(nc, tc, ctx, sel, news, usr, out_w, out_idx):
    const_pool = ctx.enter_context(tc.tile_pool(name="const", bufs=1))
    sel_pool = ctx.enter_context(tc.tile_pool(name="selp", bufs=3))
    scr_pool = ctx.enter_context(tc.tile_pool(name="scr", bufs=2))
    sq_pool = ctx.enter_context(tc.tile_pool(name="sq", bufs=2))
    acc_pool = ctx.enter_context(tc.tile_pool(name="acc", bufs=1))
    usr_pool = ctx.enter_context(tc.tile_pool(name="usrp", bufs=1))
    fin_pool = ctx.enter_context(tc.tile_pool(name="fin", bufs=2))
    psum_pool = ctx.enter_context(tc.tile_pool(name="psum", bufs=2, space="PSUM"))
    gat_pool = ctx.enter_context(tc.tile_pool(name="gat", bufs=6))
    dram_pool = ctx.enter_context(tc.tile_pool(name="dscr", bufs=1, space="DRAM"))

    # ---- constants ----
    identity = const_pool.tile([128, 128], f32)
    make_identity(nc, identity[:])
    ones_row = const_pool.tile([1, 128], f32)
    nc.vector.memset(ones_row[:], 1.0)
    # base[h, b] = (b*H + h) * L  (global row base of (b,h)'s news table)
    baseh = const_pool.tile([H, BL], i32)
    for b in range(BL):
        nc.gpsimd.iota(
            baseh[:, b : b + 1], pattern=[[1, 1]], base=b * H * L,
            channel_multiplier=L,
        )

    # ---- user vector: normalize + broadcast to 128 partitions ----
    usr_sb = usr_pool.tile([BL, D], f32)
    nc.sync.dma_start(out=usr_sb[:], in_=usr[:, :])
    usq_scr = usr_pool.tile([BL, D], f32)
    usq = usr_pool.tile([BL, 1], f32)
    nc.scalar.activation(usq_scr[:], usr_sb[:], AF.Square, accum_out=usq[:])
    unrm = usr_pool.tile([BL, 1], f32)
    nc.scalar.activation(unrm[:], usq[:], AF.Sqrt)
    unrm2 = usr_pool.tile([BL, 1], f32)
    nc.vector.tensor_scalar_max(unrm2[:], unrm[:], EPS)
    uinv = usr_pool.tile([BL, 1], f32)
    nc.vector.reciprocal(uinv[:], unrm2[:])
    usr_n = usr_pool.tile([BL, D], f32)
    nc.vector.tensor_scalar_mul(usr_n[:], usr_sb[:], uinv[:, :1])
    usr_bc = usr_pool.tile([128, BL * D], f32)
    for b in range(BL):
        stage = usr_pool.tile([1, D], f32, name=f"ustage{b}")
        nc.sync.dma_start(out=stage[:], in_=usr_n[b : b + 1, :])
        pu = psum_pool.tile([128, D], f32)
        nc.tensor.matmul(pu[:], lhsT=ones_row[:], rhs=stage[:1, :],
                         start=True, stop=True)
        nc.scalar.copy(usr_bc[:, b * D : (b + 1) * D], pu[:])

    # ---- accumulators ----
    dots = acc_pool.tile([128, NT], f32)
    ss = acc_pool.tile([128, NT], f32)

    # ---- DRAM scratch for (b,h)->(row) relayout of gather idx / weights ----
    g_scr = dram_pool.tile([BL, H, K], i32)
    w_scr = dram_pool.tile([BL, H, K], f32)

    # gather tile t (rows t*128..t*128+127 of out_w) is ready once batch
    # bmax(t) = ((t+1)*128-1)//800 has written its g/w scratch
    NTILES = NG // 128
    gather_ready = {b: [] for b in range(BL)}
    for t in range(NTILES):
        gather_ready[((t + 1) * 128 - 1) // (H * K)].append(t)
    g_flat = g_scr[:].rearrange("a b c -> (a b c)")
    w_flat = w_scr[:].rearrange("a b c -> (a b c)")

    def emit_gather(t):
        rs = slice(t * 128, (t + 1) * 128)
        gidx = gat_pool.tile([128, 1], i32, tag="gidx")
        nc.sync.dma_start(out=gidx[:], in_=g_flat[rs, None])
        wcol = gat_pool.tile([128, 1], f32, tag="wcol")
        nc.sync.dma_start(out=wcol[:], in_=w_flat[rs, None])
        ng = gat_pool.tile([128, D], f32, tag="ng")
        nc.gpsimd.indirect_dma_start(
            out=ng[:], out_offset=None, in_=news[:, :],
            in_offset=bass.IndirectOffsetOnAxis(ap=gidx[:, :1], axis=0),
        )
        ot = gat_pool.tile([128, D], f32, tag="ot")
        nc.vector.tensor_scalar_mul(ot[:], ng[:], wcol[:, :1])
        nc.sync.dma_start(out=out_w[rs, :], in_=ot[:])

    for b in range(BL):
        ub = usr_bc[:, b * D : (b + 1) * D]
        # previous batch's gather tiles, spread across this batch's chunks
        pending = list(gather_ready[b - 1]) if b > 0 else []
        for hc in range(H // CH):
            h0 = hc * CH
            st = sel_pool.tile([128, CH * D], f32)
            src = sel[b * H + h0 : b * H + h0 + CH, :, :]
            nc.sync.dma_start(
                out=st[:].rearrange("p (c d) -> p c d", c=CH),
                in_=src.rearrange("c p d -> p c d"),
            )
            if hc >= 1 and pending:
                for _ in range(2):
                    if pending:
                        emit_gather(pending.pop(0))
            for c in range(CH):
                col = b * H + h0 + c
                stc = st[:, c * D : (c + 1) * D]
                dscr = scr_pool.tile([128, D], f32)
                nc.vector.scalar_tensor_tensor(
                    out=dscr[:], in0=stc, scalar=0.0, in1=ub,
                    op0=OP.bypass, op1=OP.mult,
                    accum_out=dots[:, col : col + 1],
                )
                sscr = sq_pool.tile([128, D], f32)
                # ~10 of 200 squares on DVE to balance engine load
                if c == 4 and (b * 5 + hc) % 2 == 0:
                    nc.vector.scalar_tensor_tensor(
                        out=sscr[:], in0=stc, scalar=0.0, in1=stc,
                        op0=OP.bypass, op1=OP.mult,
                        accum_out=ss[:, col : col + 1],
                    )
                else:
                    nc.scalar.activation(
                        sscr[:], stc, AF.Square, accum_out=ss[:, col : col + 1]
                    )
        for t in pending:
            emit_gather(t)

        # ---- finish batch b: scores, transpose, top-16 ----
        bcol = slice(b * H, (b + 1) * H)
        nrm = fin_pool.tile([128, H], f32)
        nc.scalar.activation(nrm[:], ss[:, bcol], AF.Sqrt)
        nrm2 = fin_pool.tile([128, H], f32)
        nc.vector.tensor_scalar_max(nrm2[:], nrm[:], EPS)
        inv = fin_pool.tile([128, H], f32)
        nc.vector.reciprocal(inv[:], nrm2[:])
        sc = fin_pool.tile([128, H], f32)
        nc.vector.tensor_mul(sc[:], dots[:, bcol], inv[:])
        pt = psum_pool.tile([H, 128], f32)
        nc.tensor.transpose(out=pt[:], in_=sc[:], identity=identity[:])
        scT = fin_pool.tile([H, 128], f32)
        nc.scalar.copy(scT[:], pt[:])

        t16 = fin_pool.tile([H, K], f32)
        i16 = fin_pool.tile([H, K], u32)
        scT2 = fin_pool.tile([H, 128], f32)
        nc.vector.max(out=t16[:, 0:8], in_=scT[:])
        nc.vector.max_index(out=i16[:, 0:8], in_max=t16[:, 0:8], in_values=scT[:])
        nc.vector.match_replace(
            out=scT2[:], in_to_replace=t16[:, 0:8], in_values=scT[:], imm_value=NEG
        )
        nc.vector.max(out=t16[:, 8:16], in_=scT2[:])
        nc.vector.max_index(out=i16[:, 8:16], in_max=t16[:, 8:16], in_values=scT2[:])

        w16 = fin_pool.tile([H, K], f32)
        nc.vector.scalar_tensor_tensor(
            out=w16[:], in0=t16[:], scalar=THR, in1=t16[:],
            op0=OP.is_ge, op1=OP.mult,
        )
        i16c = fin_pool.tile([H, K], i32)
        nc.vector.tensor_copy(i16c[:], i16[:])
        g16 = fin_pool.tile([H, K], i32)
        nc.vector.tensor_tensor(
            out=g16[:], in0=i16c[:], in1=baseh[:, b : b + 1].to_broadcast([H, K]),
            op=OP.add,
        )

        nc.sync.dma_start(out=out_idx[b * H : (b + 1) * H, :], in_=i16c[:])
        nc.sync.dma_start(out=g_scr[b], in_=g16[:])
        nc.sync.dma_start(out=w_scr[b], in_=w16[:])

    # ---- trailing gather tiles (last batch) ----
    for t in gather_ready[BL - 1]:
        emit_gather(t)


def build_program():
    from contextlib import ExitStack

    nc = bacc.Bacc("TRN2", target_bir_lowering=False, debug=False)
    sel = nc.dram_tensor("sel", [NT, L, D], f32, kind="ExternalInput")
    news = nc.dram_tensor("news", [NT * L, D], f32, kind="ExternalInput")
    usr = nc.dram_tensor("usr", [BL, D], f32, kind="ExternalInput")
    out_w = nc.dram_tensor("out_w", [NG, D], f32, kind="ExternalOutput")
    out_idx = nc.dram_tensor("out_idx", [NT, K], i32, kind="ExternalOutput")
    with tile.TileContext(nc) as tc:
        with ExitStack() as ctx:
            _emit(nc, tc, ctx, sel, news, usr, out_w, out_idx)
    nc.finalize()
    return nc


_CACHE = {}


def _get_program():
    if "nc" not in _CACHE:
        _CACHE["nc"] = build_program()
    return _CACHE["nc"]


def make_in_maps(sel, news, usr):
    in_maps = []
    for i in range(NCORES):
        bsl = slice(i * BL, (i + 1) * BL)
        in_maps.append(
            {
                "sel": np.ascontiguousarray(sel[bsl].reshape(NT, L, D)),
                "news": np.ascontiguousarray(news[bsl].reshape(NT * L, D)),
                "usr": np.ascontiguousarray(usr[bsl].reshape(BL, D)),
            }
        )
    return in_maps


def kernel(news_selection_embedding, news_embedding, user_repr, k, threshold):
    assert int(k) == K, f"kernel hardcoded for k={K}, got {k}"
    assert abs(float(threshold) - THR) < 1e-9, f"threshold {threshold} != {THR}"
    sel = np.ascontiguousarray(np.asarray(news_selection_embedding, np.float32))
    news = np.ascontiguousarray(np.asarray(news_embedding, np.float32))
    usr = np.ascontiguousarray(np.asarray(user_repr, np.float32))
    assert sel.shape == (B, H, L, D) and news.shape == (B, H, L, D)

    nc = _get_program()
    res = run_bass_kernel_spmd(nc, make_in_maps(sel, news, usr),
                               list(range(NCORES))).results
    w = np.concatenate(
        [res[i]["out_w"].reshape(BL, H, K, D) for i in range(NCORES)], axis=0
    )
    idx = np.concatenate(
        [res[i]["out_idx"].reshape(BL, H, K) for i in range(NCORES)], axis=0
    )
    return w.astype(np.float32, copy=False), idx.astype(np.int32, copy=False)


# revision 14
# speedup vs baseline: 56.3228x; 56.3228x over previous
"""Trainium2 Bass kernel for nn_DRM_Matching (topk_masking).

Reference semantics (per batch b, history h):
    scores[l] = <sel[b,h,l]/||sel[b,h,l]||, usr[b]/||usr[b]||>   (l = 0..127)
    vals, ids = top_k(scores, 16)            # descending
    out_w[b,h,j] = news[b,h,ids[j]] * (vals[j] if vals[j] >= thr else 0)
    out_idx[b,h,j] = ids[j]

Sharding: pure data-parallel over batch, 4 batches per core on 8 cores.

Per-core pipeline:
  - stream sel tiles [128L x 256D] from HBM (1.3 MB chunks)
  - DVE: dot(sel, usr_n) per row via scalar_tensor_tensor accum
  - ACT: sum(sel^2) per row via activation(Square, accum_out)
  - per batch: scores = dot / max(sqrt(ss), eps); PE-transpose to [50H, 128L];
    top-16 via DVE max8/max_index/match_replace (two rounds)
  - weights/global gather indices bounce through DRAM scratch to re-layout
    from [50,16] to [128,1] per-partition columns
  - gpsimd indirect DMA gathers only the 16/128 needed news rows; DVE applies
    weights; DMA out
"""

import numpy as np

import concourse.bacc as bacc
import concourse.bass as bass
import concourse.mybir as mybir
import concourse.tile as tile
from concourse.bass_utils import run_bass_kernel_spmd
from concourse.masks import make_identity

# Problem geometry (hardcoded per spec)
B, H, L, D = 32, 50, 128, 256
K = 16
THR = 0.1
EPS = 1e-12
NCORES = 8
BL = B // NCORES          # 4 local batches per core
NT = BL * H               # 200 (b,h) rows per core
NG = NT * K               # 3200 gathered rows per core
CH = 10                   # histories per sel DMA chunk (1.31 MB per DMA)
NEG = -1.0e30             # replacement value for found maxima

f32 = mybir.dt.float32
i32 = mybir.dt.int32
u32 = mybir.dt.uint32
AF = mybir.ActivationFunctionType
OP = mybir.AluOpType


def _emit(nc, tc, ctx, sel, news, usr, out_w, out_idx):
    const_pool = ctx.enter_context(tc.tile_pool(name="const", bufs=1))
    sel_pool = ctx.enter_context(tc.tile_pool(name="selp", bufs=3))
    scr_pool = ctx.enter_context(tc.tile_pool(name="scr", bufs=2))
    sq_pool = ctx.enter_context(tc.tile_pool(name="sq", bufs=2))
    acc_pool = ctx.enter_context(tc.tile_pool(name="acc", bufs=1))
    usr_pool = ctx.enter_context(tc.tile_pool(name="usrp", bufs=1))
    fin_pool = ctx.enter_context(tc.tile_pool(name="fin", bufs=2))
    psum_pool = ctx.enter_context(tc.tile_pool(name="psum", bufs=2, space="PSUM"))
    gat_pool = ctx.enter_context(tc.tile_pool(name="gat", bufs=6))
    dram_pool = ctx.enter_context(tc.tile_pool(name="dscr", bufs=1, space="DRAM"))

    # ---- constants ----
    identity = const_pool.tile([128, 128], f32)
    make_identity(nc, identity[:])
    ones_row = const_pool.tile([1, 128], f32)
    nc.vector.memset(ones_row[:], 1.0)
    # base[h, b] = (b*H + h) * L  (global row base of (b,h)'s news table)
    baseh = const_pool.tile([H, BL], i32)
    for b in range(BL):
        nc.gpsimd.iota(
            baseh[:, b : b + 1], pattern=[[1, 1]], base=b * H * L,
            channel_multiplier=L,
        )

    # ---- user vector: normalize + broadcast to 128 partitions ----
    usr_sb = usr_pool.tile([BL, D], f32)
    nc.sync.dma_start(out=usr_sb[:], in_=usr[:, :])
    usq_scr = usr_pool.tile([BL, D], f32)
    usq = usr_pool.tile([BL, 1], f32)
    nc.scalar.activation(usq_scr[:], usr_sb[:], AF.Square, accum_out=usq[:])
    unrm = usr_pool.tile([BL, 1], f32)
    nc.scalar.activation(unrm[:], usq[:], AF.Sqrt)
    unrm2 = usr_pool.tile([BL, 1], f32)
    nc.vector.tensor_scalar_max(unrm2[:], unrm[:], EPS)
    uinv = usr_pool.tile([BL, 1], f32)
    nc.vector.reciprocal(uinv[:], unrm2[:])
    usr_n = usr_pool.tile([BL, D], f32)
    nc.vector.tensor_scalar_mul(usr_n[:], usr_sb[:], uinv[:, :1])
    usr_bc = usr_pool.tile([128, BL * D], f32)
    for b in range(BL):
        stage = usr_pool.tile([1, D], f32, name=f"ustage{b}")
        nc.sync.dma_start(out=stage[:], in_=usr_n[b : b + 1, :])
        pu = psum_pool.tile([128, D], f32)
        nc.tensor.matmul(pu[:], lhsT=ones_row[:], rhs=stage[:1, :],
                         start=True, stop=True)
        nc.scalar.copy(usr_bc[:, b * D : (b + 1) * D], pu[:])

    # ---- accumulators ----
    dots = acc_pool.tile([128, NT], f32)
    ss = acc_pool.tile([128, NT], f32)

    # ---- DRAM scratch for (b,h)->(row) relayout of gather idx / weights ----
    g_scr = dram_pool.tile([BL, H, K], i32)
    w_scr = dram_pool.tile([BL, H, K], f32)

    # gather tile t (rows t*128..t*128+127 of out_w) is ready once batch
    # bmax(t) = ((t+1)*128-1)//800 has written its g/w scratch
    NTILES = NG // 128
    gather_ready = {b: [] for b in range(BL)}
    for t in range(NTILES):
        gather_ready[((t + 1) * 128 - 1) // (H * K)].append(t)
    g_flat = g_scr[:].rearrange("a b c -> (a b c)")
    w_flat = w_scr[:].rearrange("a b c -> (a b c)")

    def emit_gather(t):
        rs = slice(t * 128, (t + 1) * 128)
        gidx = gat_pool.tile([128, 1], i32, tag="gidx")
        nc.sync.dma_start(out=gidx[:], in_=g_flat[rs, None])
        wcol = gat_pool.tile([128, 1], f32, tag="wcol")
        nc.sync.dma_start(out=wcol[:], in_=w_flat[rs, None])
        ng = gat_pool.tile([128, D], f32, tag="ng")
        nc.gpsimd.indirect_dma_start(
            out=ng[:], out_offset=None, in_=news[:, :],
            in_offset=bass.IndirectOffsetOnAxis(ap=gidx[:, :1], axis=0),
        )
        ot = gat_pool.tile([128, D], f32, tag="ot")
        nc.vector.tensor_scalar_mul(ot[:], ng[:], wcol[:, :1])
        nc.sync.dma_start(out=out_w[rs, :], in_=ot[:])

    for b in range(BL):
        ub = usr_bc[:, b * D : (b + 1) * D]
        # previous batch's gather tiles, spread across this batch's chunks
        pending = list(gather_ready[b - 1]) if b > 0 else []
        for hc in range(H // CH):
            h0 = hc * CH
            st = sel_pool.tile([128, CH * D], f32)
            src = sel[b * H + h0 : b * H + h0 + CH, :, :]
            nc.sync.dma_start(
                out=st[:].rearrange("p (c d) -> p c d", c=CH),
                in_=src.rearrange("c p d -> p c d"),
            )
            if hc >= 1 and pending:
                for _ in range(2):
                    if pending:
                        emit_gather(pending.pop(0))
            for c in range(CH):
                col = b * H + h0 + c
                stc = st[:, c * D : (c + 1) * D]
                dscr = scr_pool.tile([128, D], f32)
                nc.vector.scalar_tensor_tensor(
                    out=dscr[:], in0=stc, scalar=0.0, in1=ub,
                    op0=OP.bypass, op1=OP.mult,
                    accum_out=dots[:, col : col + 1],
                )
                sscr = sq_pool.tile([128, D], f32)
                # ~10 of 200 squares on DVE to balance engine load
                if c == 4 and (b * 5 + hc) % 2 == 0:
                    nc.vector.scalar_tensor_tensor(
                        out=sscr[:], in0=stc, scalar=0.0, in1=stc,
                        op0=OP.bypass, op1=OP.mult,
                        accum_out=ss[:, col : col + 1],
                    )
                else:
                    nc.scalar.activation(
                        sscr[:], stc, AF.Square, accum_out=ss[:, col : col + 1]
                    )
        for t in pending:
            emit_gather(t)

        # ---- finish batch b: scores, transpose, top-16 ----
        bcol = slice(b * H, (b + 1) * H)
        nrm = fin_pool.tile([128, H], f32)
        nc.scalar.activation(nrm[:], ss[:, bcol], AF.Sqrt)
        nrm2 = fin_pool.tile([128, H], f32)
        nc.vector.tensor_scalar_max(nrm2[:], nrm[:], EPS)
        inv = fin_pool.tile([128, H], f32)
        nc.vector.reciprocal(inv[:], nrm2[:])
        sc = fin_pool.tile([128, H], f32)
        nc.vector.tensor_mul(sc[:], dots[:, bcol], inv[:])
        pt = psum_pool.tile([H, 128], f32)
        nc.tensor.transpose(out=pt[:], in_=sc[:], identity=identity[:])
        scT = fin_pool.tile([H, 128], f32)
        nc.scalar.copy(scT[:], pt[:])

        t16 = fin_pool.tile([H, K], f32)
        i16 = fin_pool.tile([H, K], u32)
        scT2 = fin_pool.tile([H, 128], f32)
        nc.vector.max(out=t16[:, 0:8], in_=scT[:])
        nc.vector.max_index(out=i16[:, 0:8], in_max=t16[:, 0:8], in_values=scT[:])
        nc.vector.match_replace(
            out=scT2[:], in_to_replace=t16[:, 0:8], in_values=scT[:], imm_value=NEG
        )
        nc.vector.max(out=t16[:, 8:16], in_=scT2[:])
        nc.vector.max_index(out=i16[:, 8:16], in_max=t16[:, 8:16], in_values=scT2[:])

        w16 = fin_pool.tile([H, K], f32)
        nc.vector.scalar_tensor_tensor(
            out=w16[:], in0=t16[:], scalar=THR, in1=t16[:],
            op0=OP.is_ge, op1=OP.mult,
        )
        i16c = fin_pool.tile([H, K], i32)
        nc.vector.tensor_copy(i16c[:], i16[:])
        g16 = fin_pool.tile([H, K], i32)
        nc.vector.tensor_tensor(
            out=g16[:], in0=i16c[:], in1=baseh[:, b : b + 1].to_broadcast([H, K]),
            op=OP.add,
        )

        nc.sync.dma_start(out=out_idx[b * H : (b + 1) * H, :], in_=i16c[:])
        nc.sync.dma_start(out=g_scr[b], in_=g16[:])
        nc.sync.dma_start(out=w_scr[b], in_=w16[:])

    # ---- trailing gather tiles (last batch) ----
    for t in gather_ready[BL - 1]:
        emit_gather(t)


def build_program(repeat=1):
    from contextlib import ExitStack

    nc = bacc.Bacc("TRN2", target_bir_lowering=False, debug=False)
    sel = nc.dram_tensor("sel", [NT, L, D], f32, kind="ExternalInput")
    news = nc.dram_tensor("news", [NT * L, D], f32, kind="ExternalInput")
    usr = nc.dram_tensor("usr", [BL, D], f32, kind="ExternalInput")
    out_w = nc.dram_tensor("out_w", [NG, D], f32, kind="ExternalOutput")
    out_idx = nc.dram_tensor("out_idx", [NT, K], i32, kind="ExternalOutput")
    with tile.TileContext(nc) as tc:
        for _ in range(repeat):
            with ExitStack() as ctx:
                _emit(nc, tc, ctx, sel, news, usr, out_w, out_idx)
    nc.finalize()
    return nc


_CACHE = {}


def _get_program():
    if "nc" not in _CACHE:
        _CACHE["nc"] = build_program()
    return _CACHE["nc"]


def make_in_maps(sel, news, usr):
    in_maps = []
    for i in range(NCORES):
        bsl = slice(i * BL, (i + 1) * BL)
        in_maps.append(
            {
                "sel": np.ascontiguousarray(sel[bsl].reshape(NT, L, D)),
                "news": np.ascontiguousarray(news[bsl].reshape(NT * L, D)),
                "usr": np.ascontiguousarray(usr[bsl].reshape(BL, D)),
            }
        )
    return in_maps


def kernel(news_selection_embedding, news_embedding, user_repr, k, threshold):
    assert int(k) == K, f"kernel hardcoded for k={K}, got {k}"
    assert abs(float(threshold) - THR) < 1e-9, f"threshold {threshold} != {THR}"
    sel = np.ascontiguousarray(np.asarray(news_selection_embedding, np.float32))
    news = np.ascontiguousarray(np.asarray(news_embedding, np.float32))
    usr = np.ascontiguousarray(np.asarray(user_repr, np.float32))
    assert sel.shape == (B, H, L, D) and news.shape == (B, H, L, D)

    nc = _get_program()
    res = run_bass_kernel_spmd(nc, make_in_maps(sel, news, usr),
                               list(range(NCORES))).results
    w = np.concatenate(
        [res[i]["out_w"].reshape(BL, H, K, D) for i in range(NCORES)], axis=0
    )
    idx = np.concatenate(
        [res[i]["out_idx"].reshape(BL, H, K) for i in range(NCORES)], axis=0
    )
    return w.astype(np.float32, copy=False), idx.astype(np.int32, copy=False)
